# revision 18
# baseline (speedup 1.0000x reference)
"""Causal self-attention (B=2, T=2048, C=1024, H=16) on 8 trn2 NeuronCores.

Sharding (Megatron-style, per spec hint):
  - phase 1/2: tensor-parallel over heads: core p owns heads {2p, 2p+1}.
    Each core computes Q^T/K^T/V^T for its 2 heads from the full x, then
    causal attention (streaming softmax without max-subtraction; the
    denominator comes from a ones-column appended to V).
  - phase 3: AllToAll redistributes the per-head attention outputs so that
    core p holds all 1024 channels for tokens [512p, 512p+512).
  - phase 4: each core computes the full output projection for its token
    slice and writes a disjoint [512, 1024] output block.

Layout choices:
  - x is pre-transposed on the host to x^T [C, B*T] so the QKV contraction
    dim (C) lands on SBUF partitions.  Weights pre-transposed likewise.
  - scores are computed transposed: S^T[k, q] = K @ Q^T, so the softmax sum
    over k is a matmul reduction (ones column in V) and P^T = exp(S^T) is
    directly the moving operand of O^T = V^T_aug @ P, accumulated in PSUM
    over k-tiles.  Normalisation (divide by the softmax denominator) happens
    once at the end via an ACT reciprocal + DMA partition-broadcast + one
    vector multiply.
  - all matmuls run in float32r (full PE rate at moving dim >= 256).
"""

import numpy as np

B, T, C, H, D = 2, 2048, 1024, 16, 64
NCORES = 8
HL = H // NCORES        # heads per core = 2
TOK = B * T             # 4096 global tokens
TSL = TOK // NCORES     # 512 tokens per core in phase 4
P = 128
CT = C // P             # 8 contraction tiles
NTCH = TOK // 512       # 8 token chunks for qkv
NQC = T // 512          # 4 q-chunks per batch
NKT = T // P            # 16 k-tiles per batch
KG = 2                  # k-tiles per exp group
SCALE = D ** -0.5

_CACHE = {}


def _build_nc():
    import concourse.bass as bass
    import concourse.mybir as mybir
    from concourse import bacc
    from concourse.tile import TileContext

    f32 = mybir.dt.float32
    f32r = mybir.dt.float32r
    AF = mybir.ActivationFunctionType
    ALU = mybir.AluOpType

    nc = bacc.Bacc(
        "TRN2", target_bir_lowering=False, debug=False, num_devices=NCORES
    )

    xT = nc.dram_tensor("xT", [C, TOK], f32r, kind="ExternalInput")
    wqkvT = nc.dram_tensor("wqkvT", [C, 3 * P], f32r, kind="ExternalInput")
    bqkv = nc.dram_tensor("bqkv", [3 * P], f32, kind="ExternalInput")
    wpT = nc.dram_tensor("wpT", [C, C], f32r, kind="ExternalInput")
    bp = nc.dram_tensor("bp", [C], f32r, kind="ExternalInput")
    tri = nc.dram_tensor("tri", [P, P], f32r, kind="ExternalInput")
    onesd = nc.dram_tensor("ones", [P, P], f32r, kind="ExternalInput")
    ident = nc.dram_tensor("ident", [P, P], f32, kind="ExternalInput")
    y = nc.dram_tensor("y", [TSL, C], f32, kind="ExternalOutput")

    def r(ap):
        return ap.bitcast(f32r)

    with TileContext(nc, num_cores=NCORES) as tc:
        from contextlib import ExitStack

        with ExitStack() as ctx:
            const = ctx.enter_context(tc.tile_pool(name="const", bufs=1))
            persist = ctx.enter_context(tc.tile_pool(name="persist", bufs=1))
            dram = ctx.enter_context(tc.tile_pool(name="dram", bufs=1, space="DRAM"))

            # ---- constants ----
            tri_sb = const.tile([P, P], f32r)
            id_sb = const.tile([P, P], f32)
            bq_sb = const.tile([P, 3], f32)
            bp_sb = const.tile([1, C], f32r)
            ones_sb = const.tile([1, P], f32r)
            nc.sync.dma_start(ones_sb[:], onesd[0:1, :])
            w_sb = const.tile([P, CT, 3 * P], f32r)      # wqkvT tiles
            wp_sb = const.tile([P, CT, C], f32r)         # W_proj^T tiles
            nc.sync.dma_start(tri_sb[:], tri[:])
            nc.sync.dma_start(id_sb[:], ident[:])
            nc.sync.dma_start(bq_sb[:], bqkv.rearrange("(et p) -> p et", p=P))
            nc.sync.dma_start(bp_sb[:], bp.rearrange("(o c) -> o c", o=1))
            nc.sync.dma_start(w_sb[:], wqkvT.rearrange("(ct p) e -> p ct e", p=P))
            nc.sync.dma_start(wp_sb[:], wpT.rearrange("(ct p) f -> p ct f", p=P))

            # ---- persistent activations ----
            # qkv^T: rows 0-127 = Q (2 heads x 64), K, V likewise
            qT = persist.tile([P, TOK], f32r)
            kT = persist.tile([P, TOK], f32r)
            vT = persist.tile([P, TOK], f32)
            # V with ones column, per (batch, k-tile): [128 tok, 2*65]
            vaug = persist.tile([P, B, NKT, 2 * 65], f32r)
            # A^T per local head (DVE lanes are per-partition, so each
            # head's data stays at partitions 0-63 end to end)
            anorm = [persist.tile([64, TOK], f32, name=f"anorm{h}")
                     for h in range(HL)]
            ddram = dram.tile([B * HL * NQC, 512], f32)  # raw denominators
            rdram = dram.tile([B * HL, T], f32)          # reciprocals (bounce)

            # ================= phase 1: qkv^T = W_qkv_slice @ x^T =========
            with tc.tile_pool(name="xslab", bufs=2) as xpool, \
                 tc.tile_pool(name="qkvps", bufs=3, space="PSUM") as qkvps:
                for tch in range(NTCH):
                    xsl = xpool.tile([P, CT, 512], f32r, tag="x")
                    nc.sync.dma_start(
                        xsl[:],
                        xT[:, tch * 512:(tch + 1) * 512]
                        .rearrange("(ct p) t -> p ct t", p=P),
                    )
                    for et, dst in enumerate((qT, kT, vT)):
                        ps = qkvps.tile([P, 512], f32, tag="qkv")
                        for ct in range(CT):
                            nc.tensor.matmul(
                                ps[:],
                                lhsT=r(w_sb[:, ct, et * P:(et + 1) * P]),
                                rhs=r(xsl[:, ct, :]),
                                start=(ct == 0),
                                stop=(ct == CT - 1),
                            )
                        # PSUM -> SBUF with per-partition bias add
                        nc.vector.tensor_scalar_add(
                            dst[:, tch * 512:(tch + 1) * 512],
                            ps[:],
                            bq_sb[:, et:et + 1],
                        )

                # ---- phase 1b: V^T -> V tiles (PE transpose), append ones
                nc.sync.dma_start(
                    vaug[:, :, :, 64:2 * 65:65], onesd[:, 0:B * NKT * 2]
                )
                with tc.tile_pool(name="tps", bufs=2, space="PSUM") as tps:
                    for b in range(B):
                        for kt in range(NKT):
                            tp = tps.tile([P, P], f32, tag="t")
                            nc.tensor.transpose(
                                tp[:],
                                vT[:, b * T + kt * P: b * T + (kt + 1) * P],
                                id_sb[:],
                            )
                            # cols h*64..h*64+64 of tp -> vaug cols h*65..h*65+64
                            nc.vector.tensor_copy(
                                vaug[:, b, kt, 0:2 * 65]
                                .rearrange("p (h e) -> p h e", h=2)[:, :, 0:64],
                                tp.rearrange("p (h e) -> p h e", h=2),
                            )

            # ================= phase 2: attention =========================
            with tc.tile_pool(name="sps", bufs=2, space="PSUM") as sps, \
                 tc.tile_pool(name="ops", bufs=2, space="PSUM") as ops, \
                 tc.tile_pool(name="pT", bufs=3) as ppool, \
                 tc.tile_pool(name="ds", bufs=2) as dspool, \
                 tc.tile_pool(name="rb", bufs=2) as rbpool:
                for b in range(B):
                    for h in range(HL):
                        bh = b * HL + h
                        hp = slice(64 * h, 64 * h + 64)       # partition range
                        for qc in range(NQC):
                            q0 = qc * 512
                            nk = 4 * qc + 4                   # causal k-tiles
                            ops_t = ops.tile([65, 512], f32, tag="o")
                            for g0 in range(0, nk, KG):
                                gn = min(KG, nk - g0)
                                sp = sps.tile([P, KG * 512], f32, tag="s")
                                for j in range(gn):
                                    ki = g0 + j
                                    nc.tensor.matmul(
                                        sp[:, j * 512:(j + 1) * 512],
                                        lhsT=r(kT[hp, b * T + ki * P:
                                                   b * T + (ki + 1) * P]),
                                        rhs=r(qT[hp, b * T + q0:
                                                 b * T + q0 + 512]),
                                        start=True,
                                        stop=True,
                                    )
                                pt = ppool.tile([P, KG * 512], f32r, tag="p")
                                nc.scalar.activation(
                                    pt[:, 0:gn * 512],
                                    sp[:, 0:gn * 512],
                                    AF.Exp,
                                    scale=SCALE,
                                )
                                for j in range(gn):
                                    ki = g0 + j
                                    off = ki * P - q0
                                    if 0 <= off:
                                        # triangular mask on the diagonal block
                                        nc.vector.tensor_tensor(
                                            pt[:, j * 512 + off:
                                               j * 512 + off + P],
                                            pt[:, j * 512 + off:
                                               j * 512 + off + P],
                                            tri_sb[:],
                                            ALU.mult,
                                        )
                                    lo = max(0, off)
                                    nc.tensor.matmul(
                                        ops_t[:, lo:512],
                                        lhsT=r(vaug[:, b, ki,
                                                    h * 65:h * 65 + 65]),
                                        rhs=r(pt[:, j * 512 + lo:
                                                 (j + 1) * 512]),
                                        start=(ki == 0),
                                        stop=(ki == nk - 1),
                                    )
                            # stash unnormalised O^T rows + denominator row
                            nc.vector.tensor_copy(
                                anorm[h][:, b * T + q0:b * T + q0 + 512],
                                ops_t[0:64, :],
                            )
                            dst = dspool.tile([65, 512], f32, tag="ds")
                            nc.vector.tensor_copy(
                                dst[64:65, :], ops_t[64:65, :]
                            )
                            nc.sync.dma_start(
                                ddram[bh * NQC + qc:bh * NQC + qc + 1, :],
                                dst[64:65, :],
                            )

                # ---- normalisation ----
                # pack the 4x[1,2048] denominator rows into [128,64] so the
                # (per-lane-serial) reciprocal runs on all 128 lanes, then
                # unpack back to rows for the partition-broadcast DMA.
                dpack = rbpool.tile([P, 64], f32, tag="dpack", bufs=1)
                rpack = rbpool.tile([P, 64], f32, tag="rpack", bufs=1)
                rscr = rbpool.tile([P, 64], f32, tag="rscr", bufs=1)
                nc.sync.dma_start(
                    dpack[:],
                    ddram.rearrange("u (rr f) -> (u rr) f", f=64),
                )
                nc.vector.reciprocal_approx_accurate(
                    rpack[:], dpack[:], rscr[:]
                )
                nc.sync.dma_start(
                    rdram.rearrange("bh (qc rr f) -> (bh qc rr) f", rr=8, f=64),
                    rpack[:],
                )
                for b in range(B):
                    for h in range(HL):
                        bh = b * HL + h
                        rb = rbpool.tile([64, T], f32, tag="rb")
                        nc.sync.dma_start(
                            rb[:],
                            rdram[bh:bh + 1, :].to_broadcast((64, T)),
                        )
                        nc.vector.tensor_tensor(
                            anorm[h][:, b * T:(b + 1) * T],
                            anorm[h][:, b * T:(b + 1) * T],
                            rb[:],
                            ALU.mult,
                        )

            # ================= phase 3: AllToAll ==========================
            a2a_in = dram.tile([NCORES * P, TSL], f32r)
            a2a_out = dram.tile([NCORES * P, TSL], f32r)
            a2a_v = a2a_in.rearrange("(j ee) t -> ee j t", j=NCORES)
            for h in range(HL):
                nc.sync.dma_start(
                    a2a_v[64 * h:64 * h + 64],
                    anorm[h].rearrange("e (j t) -> e j t", j=NCORES)
                    .bitcast(f32r),
                )
            nc.gpsimd.collective_compute(
                "AllToAll",
                ALU.bypass,
                replica_groups=[list(range(NCORES))],
                ins=[a2a_in.opt()],
                outs=[a2a_out.opt()],
            )

            # ================= phase 4: output projection =================
            with tc.tile_pool(name="afull", bufs=1) as apool, \
                 tc.tile_pool(name="ysb", bufs=2) as ypool, \
                 tc.tile_pool(name="pps", bufs=4, space="PSUM") as pps:
                afull = apool.tile([P, NCORES, TSL], f32r)
                nc.sync.dma_start(
                    afull[:],
                    a2a_out.rearrange("(i e) t -> e i t", i=NCORES),
                )
                for tt in range(TSL // P):
                    for fc in range(C // 512):
                        ps = pps.tile([P, 512], f32, tag="y")
                        # bias via rank-1 matmul (ones row (x) bias row)
                        nc.tensor.matmul(
                            ps[:],
                            lhsT=r(ones_sb[:]),
                            rhs=r(bp_sb[:, fc * 512:(fc + 1) * 512]),
                            start=True,
                            stop=False,
                        )
                        for i in range(NCORES):
                            nc.tensor.matmul(
                                ps[:],
                                lhsT=r(afull[:, i, tt * P:(tt + 1) * P]),
                                rhs=r(wp_sb[:, i, fc * 512:(fc + 1) * 512]),
                                start=False,
                                stop=(i == NCORES - 1),
                            )
                        ysb = ypool.tile([P, 512], f32, tag="ysb")
                        nc.vector.tensor_copy(ysb[:], ps[:])
                        nc.sync.dma_start(
                            y[tt * P:(tt + 1) * P, fc * 512:(fc + 1) * 512],
                            ysb[:],
                        )
    nc.compile()
    return nc


def _prep_inputs(x, W_qkv, b_qkv, W_proj, b_proj):
    x = np.asarray(x, dtype=np.float32)
    W_qkv = np.asarray(W_qkv, dtype=np.float32)
    b_qkv = np.asarray(b_qkv, dtype=np.float32)
    W_proj = np.asarray(W_proj, dtype=np.float32)
    b_proj = np.asarray(b_proj, dtype=np.float32)

    xT = np.ascontiguousarray(x.reshape(TOK, C).T)
    wpT = np.ascontiguousarray(W_proj.T)
    tri = np.triu(np.ones((P, P), dtype=np.float32))
    ident = np.eye(P, dtype=np.float32)
    ones = np.ones((P, P), dtype=np.float32)

    in_maps = []
    for p in range(NCORES):
        rows = np.r_[128 * p:128 * p + 128,
                     C + 128 * p:C + 128 * p + 128,
                     2 * C + 128 * p:2 * C + 128 * p + 128]
        wslice = W_qkv[rows]                      # [384, 1024]
        bslice = np.ascontiguousarray(b_qkv[rows])
        in_maps.append({
            "xT": xT,
            "wqkvT": np.ascontiguousarray(wslice.T),
            "bqkv": bslice,
            "wpT": wpT,
            "bp": b_proj,
            "tri": tri,
            "ident": ident,
            "ones": ones,
        })
    return in_maps


def kernel(x, W_qkv, b_qkv, W_proj, b_proj, _trace=False):
    from concourse import bass_utils

    if "nc" not in _CACHE:
        _CACHE["nc"] = _build_nc()
    nc = _CACHE["nc"]
    in_maps = _prep_inputs(x, W_qkv, b_qkv, W_proj, b_proj)
    res = bass_utils.run_bass_kernel_spmd(
        nc, in_maps, core_ids=list(range(NCORES)), trace=_trace,
    )
    _CACHE["last_result"] = res
    yfull = np.concatenate([r["y"] for r in res.results], axis=0)
    return yfull.reshape(B, T, C).astype(np.float32)


# revision 20
# speedup vs baseline: 1.1496x; 1.1496x over previous
"""Causal self-attention (B=2, T=2048, C=1024, H=16) on 8 trn2 NeuronCores.

Sharding (Megatron-style, per spec hint):
  - phase 1/2: tensor-parallel over heads: core p owns heads {2p, 2p+1}.
    Each core computes Q^T/K^T/V^T for its 2 heads from the full x, then
    causal attention (streaming softmax without max-subtraction; the
    denominator comes from a ones-column appended to V).
  - per batch: an AllToAll redistributes that batch's attention outputs so
    that core p holds all 1024 channels for the batch's tokens
    [256p, 256p+256); batch 0's AllToAll and projection overlap batch 1's
    attention on the compute engines.
  - projection: each core computes the full output projection for its two
    256-token slices and writes a disjoint [512, 1024] output block
    (rows b*256+i = batch b, token 256*p+i).

Layout choices:
  - x is pre-transposed on the host to x^T [C, B*T] so the QKV contraction
    dim (C) lands on SBUF partitions.  Weights pre-transposed likewise.
  - scores are computed transposed: S^T[k, q] = K @ Q^T, so the softmax sum
    over k is a matmul reduction (ones column in V) and P^T = exp(S^T) is
    directly the moving operand of O^T = V^T_aug @ P, accumulated in PSUM
    over k-tiles.  Normalisation (divide by the softmax denominator) is a
    packed reciprocal + DMA partition-broadcast + one vector multiply,
    pipelined per batch.
  - all matmuls run in float32r (full PE rate at moving dim >= 256).
"""

import numpy as np

B, T, C, H, D = 2, 2048, 1024, 16, 64
NCORES = 8
HL = H // NCORES        # heads per core = 2
TOK = B * T             # 4096 global tokens
TSL = TOK // NCORES     # 512 output tokens per core (256 per batch)
SL = 256                # per-batch token slice per core
P = 128
CT = C // P             # 8 contraction tiles
NTCH = TOK // 512       # 8 token chunks for qkv
NQC = T // 512          # 4 q-chunks per batch
NKT = T // P            # 16 k-tiles per batch
KG = 2                  # k-tiles per exp group
SCALE = D ** -0.5

_CACHE = {}


def _build_nc():
    import concourse.bass as bass
    import concourse.mybir as mybir
    from concourse import bacc
    from concourse.tile import TileContext

    f32 = mybir.dt.float32
    f32r = mybir.dt.float32r
    AF = mybir.ActivationFunctionType
    ALU = mybir.AluOpType

    nc = bacc.Bacc(
        "TRN2", target_bir_lowering=False, debug=False, num_devices=NCORES
    )

    xT = nc.dram_tensor("xT", [C, TOK], f32r, kind="ExternalInput")
    wqkvT = nc.dram_tensor("wqkvT", [C, 3 * P], f32r, kind="ExternalInput")
    bqkv = nc.dram_tensor("bqkv", [3 * P], f32, kind="ExternalInput")
    wpT = nc.dram_tensor("wpT", [C, C], f32r, kind="ExternalInput")
    bp = nc.dram_tensor("bp", [C], f32r, kind="ExternalInput")
    tri = nc.dram_tensor("tri", [P, P], f32r, kind="ExternalInput")
    onesd = nc.dram_tensor("ones", [P, P], f32r, kind="ExternalInput")
    ident = nc.dram_tensor("ident", [P, P], f32, kind="ExternalInput")
    y = nc.dram_tensor("y", [TSL, C], f32, kind="ExternalOutput")

    with TileContext(nc, num_cores=NCORES) as tc:
        from contextlib import ExitStack

        with ExitStack() as ctx:
            const = ctx.enter_context(tc.tile_pool(name="const", bufs=1))
            persist = ctx.enter_context(tc.tile_pool(name="persist", bufs=1))
            dram = ctx.enter_context(tc.tile_pool(name="dram", bufs=1, space="DRAM"))

            # ---- constants (wp_sb is DMA'd late, before the projection) ----
            tri_sb = const.tile([P, P], f32r)
            id_sb = const.tile([P, P], f32)
            bq_sb = const.tile([P, 3], f32)
            bp_sb = const.tile([1, C], f32r)
            ones_sb = const.tile([1, P], f32r)
            w_sb = const.tile([P, CT, 3 * P], f32r)     # wqkvT tiles
            wp_sb = const.tile([P, CT, C], f32r)        # W_proj^T tiles
            nc.sync.dma_start(w_sb[:], wqkvT.rearrange("(ct p) e -> p ct e", p=P))
            nc.sync.dma_start(tri_sb[:], tri[:])
            nc.sync.dma_start(id_sb[:], ident[:])
            nc.sync.dma_start(bq_sb[:], bqkv.rearrange("(et p) -> p et", p=P))
            nc.sync.dma_start(bp_sb[:], bp.rearrange("(o c) -> o c", o=1))
            nc.sync.dma_start(ones_sb[:], onesd[0:1, :])

            # ---- persistent activations ----
            qT = persist.tile([P, TOK], f32r)
            kT = persist.tile([P, TOK], f32r)
            vT = persist.tile([P, TOK], f32)
            # V with ones column, per (batch, k-tile): [128 tok, 2*65]
            vaug = persist.tile([P, B, NKT, 2 * 65], f32r)
            # A^T per local head (DVE lanes are per-partition, so each
            # head's data stays at partitions 0-63 end to end)
            anorm = [persist.tile([64, TOK], f32, name=f"anorm{h}")
                     for h in range(HL)]
            ddram = dram.tile([B * HL * NQC, 512], f32)  # raw denominators
            rdram = dram.tile([B * HL, T], f32)          # reciprocals (bounce)

            # ================= phase 1: qkv^T = W_qkv_slice @ x^T =========
            with tc.tile_pool(name="xslab", bufs=2) as xpool, \
                 tc.tile_pool(name="qkvps", bufs=3, space="PSUM") as qkvps:
                for tch in range(NTCH):
                    xsl = xpool.tile([P, CT, 512], f32r, tag="x")
                    nc.sync.dma_start(
                        xsl[:],
                        xT[:, tch * 512:(tch + 1) * 512]
                        .rearrange("(ct p) t -> p ct t", p=P),
                    )
                    for et, dst in enumerate((qT, kT, vT)):
                        ps = qkvps.tile([P, 512], f32, tag="qkv")
                        for ct in range(CT):
                            nc.tensor.matmul(
                                ps[:],
                                lhsT=w_sb[:, ct, et * P:(et + 1) * P],
                                rhs=xsl[:, ct, :],
                                start=(ct == 0),
                                stop=(ct == CT - 1),
                            )
                        # PSUM -> SBUF with per-partition bias add
                        nc.vector.tensor_scalar_add(
                            dst[:, tch * 512:(tch + 1) * 512],
                            ps[:],
                            bq_sb[:, et:et + 1],
                        )

                # ---- phase 1b: V^T -> V tiles (PE transpose), append ones
                nc.sync.dma_start(
                    vaug[:, :, :, 64:2 * 65:65], onesd[:, 0:B * NKT * 2]
                )
                with tc.tile_pool(name="tps", bufs=2, space="PSUM") as tps:
                    for b in range(B):
                        for kt in range(NKT):
                            tp = tps.tile([P, P], f32, tag="t")
                            nc.tensor.transpose(
                                tp[:],
                                vT[:, b * T + kt * P: b * T + (kt + 1) * P],
                                id_sb[:],
                            )
                            # cols h*64..h*64+64 of tp -> vaug cols h*65+
                            nc.vector.tensor_copy(
                                vaug[:, b, kt, 0:2 * 65]
                                .rearrange("p (h e) -> p h e", h=2)[:, :, 0:64],
                                tp.rearrange("p (h e) -> p h e", h=2),
                            )

            # ============ phase 2+3+4, pipelined per batch ================
            with tc.tile_pool(name="sps", bufs=2, space="PSUM") as sps, \
                 tc.tile_pool(name="ops", bufs=2, space="PSUM") as ops, \
                 tc.tile_pool(name="pT", bufs=3) as ppool, \
                 tc.tile_pool(name="ds", bufs=2) as dspool, \
                 tc.tile_pool(name="rp", bufs=2) as rppool, \
                 tc.tile_pool(name="rb", bufs=2) as rbpool, \
                 tc.tile_pool(name="afull", bufs=2) as apool, \
                 tc.tile_pool(name="ysb", bufs=2) as ypool:

                def attention(b):
                    for h in range(HL):
                        bh = b * HL + h
                        hp = slice(64 * h, 64 * h + 64)
                        for qc in range(NQC):
                            q0 = qc * 512
                            nk = 4 * qc + 4               # causal k-tiles
                            ops_t = ops.tile([65, 512], f32, tag="o")
                            for g0 in range(0, nk, KG):
                                gn = min(KG, nk - g0)
                                sp = sps.tile([P, KG * 512], f32, tag="s")
                                for j in range(gn):
                                    ki = g0 + j
                                    nc.tensor.matmul(
                                        sp[:, j * 512:(j + 1) * 512],
                                        lhsT=kT[hp, b * T + ki * P:
                                                b * T + (ki + 1) * P],
                                        rhs=qT[hp, b * T + q0:
                                               b * T + q0 + 512],
                                        start=True,
                                        stop=True,
                                    )
                                pt = ppool.tile([P, KG * 512], f32r, tag="p")
                                nc.scalar.activation(
                                    pt[:, 0:gn * 512],
                                    sp[:, 0:gn * 512],
                                    AF.Exp,
                                    scale=SCALE,
                                )
                                for j in range(gn):
                                    ki = g0 + j
                                    off = ki * P - q0
                                    if 0 <= off:
                                        # triangular mask on diagonal block
                                        nc.vector.tensor_tensor(
                                            pt[:, j * 512 + off:
                                               j * 512 + off + P],
                                            pt[:, j * 512 + off:
                                               j * 512 + off + P],
                                            tri_sb[:],
                                            ALU.mult,
                                        )
                                    lo = max(0, off)
                                    nc.tensor.matmul(
                                        ops_t[:, lo:512],
                                        lhsT=vaug[:, b, ki,
                                                  h * 65:h * 65 + 65],
                                        rhs=pt[:, j * 512 + lo:
                                               (j + 1) * 512],
                                        start=(ki == 0),
                                        stop=(ki == nk - 1),
                                    )
                            # stash unnormalised O^T rows + denominator row
                            nc.vector.tensor_copy(
                                anorm[h][:, b * T + q0:b * T + q0 + 512],
                                ops_t[0:64, :],
                            )
                            dst = dspool.tile([65, 512], f32, tag="ds")
                            nc.vector.tensor_copy(
                                dst[64:65, :], ops_t[64:65, :]
                            )
                            nc.sync.dma_start(
                                ddram[bh * NQC + qc:bh * NQC + qc + 1, :],
                                dst[64:65, :],
                            )

                def normalize(b):
                    for h in range(HL):
                        bh = b * HL + h
                        dpk = rppool.tile([32, 64], f32, tag="dpk")
                        rpk = rppool.tile([32, 64], f32, tag="rpk")
                        rsc = rppool.tile([32, 64], f32, tag="rsc")
                        nc.sync.dma_start(
                            dpk[:],
                            ddram[bh * NQC:(bh + 1) * NQC, :]
                            .rearrange("u (rr f) -> (u rr) f", f=64),
                        )
                        nc.vector.reciprocal_approx_accurate(
                            rpk[:], dpk[:], rsc[:]
                        )
                        nc.sync.dma_start(
                            rdram[bh:bh + 1, :]
                            .rearrange("o (rr f) -> (o rr) f", f=64),
                            rpk[:],
                        )
                        rb = rbpool.tile([64, T], f32, tag="rb")
                        nc.sync.dma_start(
                            rb[:],
                            rdram[bh:bh + 1, :].to_broadcast((64, T)),
                        )
                        nc.vector.tensor_tensor(
                            anorm[h][:, b * T:(b + 1) * T],
                            anorm[h][:, b * T:(b + 1) * T],
                            rb[:],
                            ALU.mult,
                        )

                def a2a(b):
                    a2a_in = dram.tile([NCORES * P, SL], f32r,
                                       name=f"a2a_in{b}")
                    a2a_out = dram.tile([NCORES * P, SL], f32r,
                                        name=f"a2a_out{b}")
                    a2a_v = a2a_in.rearrange("(j ee) t -> ee j t", j=NCORES)
                    for h in range(HL):
                        nc.sync.dma_start(
                            a2a_v[64 * h:64 * h + 64],
                            anorm[h][:, b * T:(b + 1) * T]
                            .rearrange("e (j t) -> e j t", j=NCORES)
                            .bitcast(f32r),
                        )
                    nc.gpsimd.collective_compute(
                        "AllToAll",
                        ALU.bypass,
                        replica_groups=[list(range(NCORES))],
                        ins=[a2a_in.opt()],
                        outs=[a2a_out.opt()],
                    )
                    return a2a_out

                def proj(b, a2a_out):
                    afull = apool.tile([P, NCORES, SL], f32r, tag="af")
                    nc.sync.dma_start(
                        afull[:],
                        a2a_out.rearrange("(i e) t -> e i t", i=NCORES),
                    )
                    for tt in range(SL // P):
                        for fc in range(C // 512):
                            ps = pps.tile([P, 512], f32, tag="y")
                            nc.tensor.matmul(
                                ps[:],
                                lhsT=ones_sb[:],
                                rhs=bp_sb[:, fc * 512:(fc + 1) * 512],
                                start=True,
                                stop=False,
                            )
                            for i in range(NCORES):
                                nc.tensor.matmul(
                                    ps[:],
                                    lhsT=afull[:, i, tt * P:(tt + 1) * P],
                                    rhs=wp_sb[:, i, fc * 512:(fc + 1) * 512],
                                    start=False,
                                    stop=(i == NCORES - 1),
                                )
                            ysb = ypool.tile([P, 512], f32, tag="ysb")
                            nc.vector.tensor_copy(ysb[:], ps[:])
                            nc.sync.dma_start(
                                y[b * SL + tt * P:b * SL + (tt + 1) * P,
                                  fc * 512:(fc + 1) * 512],
                                ysb[:],
                            )

                with tc.tile_pool(name="pps", bufs=2, space="PSUM") as pps:
                    attention(0)
                    normalize(0)
                    out0 = a2a(0)
                    # W_proj load overlaps batch-1 attention
                    nc.sync.dma_start(
                        wp_sb[:], wpT.rearrange("(ct p) f -> p ct f", p=P)
                    )
                    attention(1)
                    proj(0, out0)
                    normalize(1)
                    out1 = a2a(1)
                    proj(1, out1)
    nc.compile()
    return nc


def _prep_inputs(x, W_qkv, b_qkv, W_proj, b_proj):
    x = np.asarray(x, dtype=np.float32)
    W_qkv = np.asarray(W_qkv, dtype=np.float32)
    b_qkv = np.asarray(b_qkv, dtype=np.float32)
    W_proj = np.asarray(W_proj, dtype=np.float32)
    b_proj = np.asarray(b_proj, dtype=np.float32)

    xT = np.ascontiguousarray(x.reshape(TOK, C).T)
    wpT = np.ascontiguousarray(W_proj.T)
    tri = np.triu(np.ones((P, P), dtype=np.float32))
    ident = np.eye(P, dtype=np.float32)
    ones = np.ones((P, P), dtype=np.float32)

    in_maps = []
    for p in range(NCORES):
        rows = np.r_[128 * p:128 * p + 128,
                     C + 128 * p:C + 128 * p + 128,
                     2 * C + 128 * p:2 * C + 128 * p + 128]
        wslice = W_qkv[rows]                      # [384, 1024]
        bslice = np.ascontiguousarray(b_qkv[rows])
        in_maps.append({
            "xT": xT,
            "wqkvT": np.ascontiguousarray(wslice.T),
            "bqkv": bslice,
            "wpT": wpT,
            "bp": b_proj,
            "tri": tri,
            "ident": ident,
            "ones": ones,
        })
    return in_maps


def kernel(x, W_qkv, b_qkv, W_proj, b_proj, _trace=False):
    from concourse import bass_utils

    if "nc" not in _CACHE:
        _CACHE["nc"] = _build_nc()
    nc = _CACHE["nc"]
    in_maps = _prep_inputs(x, W_qkv, b_qkv, W_proj, b_proj)
    res = bass_utils.run_bass_kernel_spmd(
        nc, in_maps, core_ids=list(range(NCORES)), trace=_trace,
    )
    _CACHE["last_result"] = res
    # core p rows: [b*256 + i] = batch b, token 256*p + i
    yfull = np.empty((B, T, C), dtype=np.float32)
    for p, rmap in enumerate(res.results):
        yp = rmap["y"]
        for b in range(B):
            yfull[b, SL * p:SL * (p + 1)] = yp[b * SL:(b + 1) * SL]
    return yfull


# revision 22
# speedup vs baseline: 1.3608x; 1.1837x over previous
"""Causal self-attention (B=2, T=2048, C=1024, H=16) on 8 trn2 NeuronCores.

Sharding (Megatron-style, per spec hint):
  - tensor-parallel over heads: core p owns heads {2p, 2p+1}.  Each core
    computes Q^T/K^T/V^T for its 2 heads from the full x, then causal
    attention (streaming softmax without max-subtraction; the denominator
    comes from a ones-column appended to V).
  - per batch: an AllToAll redistributes that batch's attention outputs so
    that core p holds all 1024 channels for the batch's tokens
    [256p, 256p+256); batch 0's AllToAll and projection overlap batch 1's
    qkv/attention work.
  - projection: each core computes the full output projection for its two
    256-token slices and writes a disjoint [512, 1024] output block
    (rows b*256+i = batch b, token 256*p+i).

The emission order pipelines per batch so the (in-order) PE never waits on
the 16.8 MB x^T stream: qkv(b0) -> attention(b0) -> qkv(b1) [x tail
streams during attention(b0)] -> attention(b1) -> projections.

Layouts: x/W pre-transposed on host so contractions land on partitions;
scores computed transposed (S^T = K Q^T) so the softmax sum is a matmul
reduction and exp(S^T) feeds O^T = V^T_aug P directly, accumulated in PSUM
over k-tiles; all matmuls in float32r.
"""

import numpy as np

B, T, C, H, D = 2, 2048, 1024, 16, 64
NCORES = 8
HL = H // NCORES        # heads per core = 2
TOK = B * T             # 4096 global tokens
TSL = TOK // NCORES     # 512 output tokens per core (256 per batch)
SL = 256                # per-batch token slice per core
P = 128
CT = C // P             # 8 contraction tiles
NQC = T // 512          # 4 q-chunks per batch
NKT = T // P            # 16 k-tiles per batch
KG = 2                  # k-tiles per exp group
SCALE = D ** -0.5

_CACHE = {}


def _build_nc():
    import concourse.bass as bass
    import concourse.mybir as mybir
    from concourse import bacc
    from concourse.tile import TileContext

    f32 = mybir.dt.float32
    f32r = mybir.dt.float32r
    AF = mybir.ActivationFunctionType
    ALU = mybir.AluOpType

    nc = bacc.Bacc(
        "TRN2", target_bir_lowering=False, debug=False, num_devices=NCORES
    )

    xT = nc.dram_tensor("xT", [C, TOK], f32r, kind="ExternalInput")
    wqkvT = nc.dram_tensor("wqkvT", [C, 3 * P], f32r, kind="ExternalInput")
    bqkv = nc.dram_tensor("bqkv", [3 * P], f32, kind="ExternalInput")
    wpT = nc.dram_tensor("wpT", [C, C], f32r, kind="ExternalInput")
    bp = nc.dram_tensor("bp", [C], f32r, kind="ExternalInput")
    tri = nc.dram_tensor("tri", [P, P], f32r, kind="ExternalInput")
    onesd = nc.dram_tensor("ones", [P, P], f32r, kind="ExternalInput")
    ident = nc.dram_tensor("ident", [P, P], f32, kind="ExternalInput")
    y = nc.dram_tensor("y", [TSL, C], f32, kind="ExternalOutput")

    with TileContext(nc, num_cores=NCORES) as tc:
        from contextlib import ExitStack

        with ExitStack() as ctx:
            const = ctx.enter_context(tc.tile_pool(name="const", bufs=1))
            persist = ctx.enter_context(tc.tile_pool(name="persist", bufs=1))
            dram = ctx.enter_context(tc.tile_pool(name="dram", bufs=1, space="DRAM"))

            # ---- constants; small ones first so nothing queues behind bulk
            tri_sb = const.tile([P, P], f32r)
            id_sb = const.tile([P, P], f32)
            bq_sb = const.tile([P, 3], f32)
            bp_sb = const.tile([1, C], f32r)
            ones_sb = const.tile([1, P], f32r)
            ones2_sb = const.tile([P, 2], f32)
            w_sb = const.tile([P, CT, 3 * P], f32r)     # wqkvT tiles
            wp_sb = const.tile([P, CT, C], f32r)        # W_proj^T (loaded late)
            nc.sync.dma_start(tri_sb[:], tri[:])
            nc.sync.dma_start(id_sb[:], ident[:])
            nc.sync.dma_start(bq_sb[:], bqkv.rearrange("(et p) -> p et", p=P))
            nc.sync.dma_start(bp_sb[:], bp.rearrange("(o c) -> o c", o=1))
            nc.sync.dma_start(ones_sb[:], onesd[0:1, :])
            nc.sync.dma_start(ones2_sb[:], onesd[:, 0:2].bitcast(f32))
            nc.sync.dma_start(w_sb[:], wqkvT.rearrange("(ct p) e -> p ct e", p=P))

            # ---- persistent activations (per batch for fine-grained deps)
            qTb = [persist.tile([P, T], f32r, name=f"qT{b}") for b in range(B)]
            kTb = [persist.tile([P, T], f32r, name=f"kT{b}") for b in range(B)]
            vTb = [persist.tile([P, T], f32, name=f"vT{b}") for b in range(B)]
            # V with ones column, per batch: [128 tok, k-tile, 2*65]
            vaugb = [persist.tile([P, NKT, 2 * 65], f32r, name=f"vaug{b}")
                     for b in range(B)]
            # A^T per local head (each head stays at partitions 0-63)
            anorm = [persist.tile([64, TOK], f32, name=f"anorm{h}")
                     for h in range(HL)]
            ddram = dram.tile([B * HL * NQC, 512], f32)  # raw denominators
            rdram = dram.tile([B * HL, T], f32)          # reciprocals (bounce)

            pools = [
                tc.tile_pool(name="sps", bufs=2, space="PSUM"),
                tc.tile_pool(name="ops", bufs=2, space="PSUM"),
                tc.tile_pool(name="mm", bufs=2, space="PSUM"),
                tc.tile_pool(name="pT", bufs=2),
                tc.tile_pool(name="ds", bufs=2),
                tc.tile_pool(name="rp", bufs=2),
            ]
            sps, ops, mm, ppool, dspool, rppool = (
                ctx.enter_context(p) for p in pools)

            def qkv(b):
                """qkv^T for batch b's 4 token chunks + V transposes."""
                for tc4 in range(4):
                    xsl = xpool.tile([P, CT, 512], f32r, tag="x")
                    t0 = b * T + tc4 * 512
                    nc.sync.dma_start(
                        xsl[:],
                        xT[:, t0:t0 + 512].rearrange("(ct p) t -> p ct t", p=P),
                    )
                    for et, dstl in enumerate((qTb, kTb, vTb)):
                        ps = mm.tile([P, 512], f32, tag="mm")
                        for ct in range(CT):
                            nc.tensor.matmul(
                                ps[:],
                                lhsT=w_sb[:, ct, et * P:(et + 1) * P],
                                rhs=xsl[:, ct, :],
                                start=(ct == 0),
                                stop=(ct == CT - 1),
                            )
                        nc.vector.tensor_scalar_add(
                            dstl[b][:, tc4 * 512:(tc4 + 1) * 512],
                            ps[:],
                            bq_sb[:, et:et + 1],
                        )
                    # V^T -> V for this chunk's 4 k-tiles (PE transpose)
                    for kt in range(tc4 * 4, tc4 * 4 + 4):
                        tp = mm.tile([P, P], f32, tag="mm")
                        nc.tensor.transpose(
                            tp[:],
                            vTb[b][:, kt * P:(kt + 1) * P],
                            id_sb[:],
                        )
                        nc.vector.tensor_copy(
                            vaugb[b][:, kt, 0:2 * 65]
                            .rearrange("p (h e) -> p h e", h=2)[:, :, 0:64],
                            tp.rearrange("p (h e) -> p h e", h=2),
                        )
                        nc.vector.tensor_copy(
                            vaugb[b][:, kt, 64:2 * 65:65], ones2_sb[:]
                        )

            def attention(b):
                for h in range(HL):
                    bh = b * HL + h
                    hp = slice(64 * h, 64 * h + 64)
                    for qc in range(NQC):
                        q0 = qc * 512
                        nk = 4 * qc + 4               # causal k-tiles
                        ops_t = ops.tile([65, 512], f32, tag="o")
                        for g0 in range(0, nk, KG):
                            gn = min(KG, nk - g0)
                            sp = sps.tile([P, KG * 512], f32, tag="s")
                            for j in range(gn):
                                ki = g0 + j
                                nc.tensor.matmul(
                                    sp[:, j * 512:(j + 1) * 512],
                                    lhsT=kTb[b][hp, ki * P:(ki + 1) * P],
                                    rhs=qTb[b][hp, q0:q0 + 512],
                                    start=True,
                                    stop=True,
                                )
                            pt = ppool.tile([P, KG * 512], f32r, tag="p")
                            nc.scalar.activation(
                                pt[:, 0:gn * 512],
                                sp[:, 0:gn * 512],
                                AF.Exp,
                                scale=SCALE,
                            )
                            for j in range(gn):
                                ki = g0 + j
                                off = ki * P - q0
                                if 0 <= off:
                                    nc.vector.tensor_tensor(
                                        pt[:, j * 512 + off:
                                           j * 512 + off + P],
                                        pt[:, j * 512 + off:
                                           j * 512 + off + P],
                                        tri_sb[:],
                                        ALU.mult,
                                    )
                                lo = max(0, off)
                                nc.tensor.matmul(
                                    ops_t[:, lo:512],
                                    lhsT=vaugb[b][:, ki, h * 65:h * 65 + 65],
                                    rhs=pt[:, j * 512 + lo:(j + 1) * 512],
                                    start=(ki == 0),
                                    stop=(ki == nk - 1),
                                )
                        # stash unnormalised O^T rows + denominator row
                        nc.vector.tensor_copy(
                            anorm[h][:, b * T + q0:b * T + q0 + 512],
                            ops_t[0:64, :],
                        )
                        dst = dspool.tile([65, 512], f32, tag="ds")
                        nc.vector.tensor_copy(dst[64:65, :], ops_t[64:65, :])
                        nc.sync.dma_start(
                            ddram[bh * NQC + qc:bh * NQC + qc + 1, :],
                            dst[64:65, :],
                        )

            def normalize(b):
                for h in range(HL):
                    bh = b * HL + h
                    dpk = rppool.tile([32, 64], f32, tag="dpk")
                    rpk = rppool.tile([32, 64], f32, tag="rpk")
                    rsc = rppool.tile([32, 64], f32, tag="rsc")
                    nc.sync.dma_start(
                        dpk[:],
                        ddram[bh * NQC:(bh + 1) * NQC, :]
                        .rearrange("u (rr f) -> (u rr) f", f=64),
                    )
                    nc.vector.reciprocal_approx_accurate(rpk[:], dpk[:], rsc[:])
                    nc.sync.dma_start(
                        rdram[bh:bh + 1, :]
                        .rearrange("o (rr f) -> (o rr) f", f=64),
                        rpk[:],
                    )
                    rb = rbpool.tile([64, T], f32, tag="rb")
                    nc.sync.dma_start(
                        rb[:],
                        rdram[bh:bh + 1, :].to_broadcast((64, T)),
                    )
                    nc.vector.tensor_tensor(
                        anorm[h][:, b * T:(b + 1) * T],
                        anorm[h][:, b * T:(b + 1) * T],
                        rb[:],
                        ALU.mult,
                    )

            def a2a(b):
                a2a_in = dram.tile([NCORES * P, SL], f32r, name=f"a2a_in{b}")
                a2a_out = dram.tile([NCORES * P, SL], f32r, name=f"a2a_out{b}")
                a2a_v = a2a_in.rearrange("(j ee) t -> ee j t", j=NCORES)
                for h in range(HL):
                    nc.sync.dma_start(
                        a2a_v[64 * h:64 * h + 64],
                        anorm[h][:, b * T:(b + 1) * T]
                        .rearrange("e (j t) -> e j t", j=NCORES)
                        .bitcast(f32r),
                    )
                nc.gpsimd.collective_compute(
                    "AllToAll",
                    ALU.bypass,
                    replica_groups=[list(range(NCORES))],
                    ins=[a2a_in.opt()],
                    outs=[a2a_out.opt()],
                )
                return a2a_out

            def proj(b, a2a_out):
                afull = apool.tile([P, NCORES, SL], f32r, tag="af")
                nc.sync.dma_start(
                    afull[:],
                    a2a_out.rearrange("(i e) t -> e i t", i=NCORES),
                )
                for tt in range(SL // P):
                    for fc in range(C // 512):
                        ps = mm.tile([P, 512], f32, tag="mm")
                        nc.tensor.matmul(
                            ps[:],
                            lhsT=ones_sb[:],
                            rhs=bp_sb[:, fc * 512:(fc + 1) * 512],
                            start=True,
                            stop=False,
                        )
                        for i in range(NCORES):
                            nc.tensor.matmul(
                                ps[:],
                                lhsT=afull[:, i, tt * P:(tt + 1) * P],
                                rhs=wp_sb[:, i, fc * 512:(fc + 1) * 512],
                                start=False,
                                stop=(i == NCORES - 1),
                            )
                        ysb = ypool.tile([P, 512], f32, tag="ysb")
                        nc.vector.tensor_copy(ysb[:], ps[:])
                        nc.sync.dma_start(
                            y[b * SL + tt * P:b * SL + (tt + 1) * P,
                              fc * 512:(fc + 1) * 512],
                            ysb[:],
                        )

            with tc.tile_pool(name="xslab", bufs=2) as xpool:
                qkv(0)
                attention(0)
                qkv(1)
            with tc.tile_pool(name="rb", bufs=1) as rbpool, \
                 tc.tile_pool(name="afull", bufs=2) as apool, \
                 tc.tile_pool(name="ysb", bufs=2) as ypool:
                normalize(0)
                out0 = a2a(0)
                nc.sync.dma_start(
                    wp_sb[:], wpT.rearrange("(ct p) f -> p ct f", p=P)
                )
                attention(1)
                proj(0, out0)
                normalize(1)
                out1 = a2a(1)
                proj(1, out1)
    nc.compile()
    return nc


def _prep_inputs(x, W_qkv, b_qkv, W_proj, b_proj):
    x = np.asarray(x, dtype=np.float32)
    W_qkv = np.asarray(W_qkv, dtype=np.float32)
    b_qkv = np.asarray(b_qkv, dtype=np.float32)
    W_proj = np.asarray(W_proj, dtype=np.float32)
    b_proj = np.asarray(b_proj, dtype=np.float32)

    xT = np.ascontiguousarray(x.reshape(TOK, C).T)
    wpT = np.ascontiguousarray(W_proj.T)
    tri = np.triu(np.ones((P, P), dtype=np.float32))
    ident = np.eye(P, dtype=np.float32)
    ones = np.ones((P, P), dtype=np.float32)

    in_maps = []
    for p in range(NCORES):
        rows = np.r_[128 * p:128 * p + 128,
                     C + 128 * p:C + 128 * p + 128,
                     2 * C + 128 * p:2 * C + 128 * p + 128]
        wslice = W_qkv[rows]                      # [384, 1024]
        bslice = np.ascontiguousarray(b_qkv[rows])
        in_maps.append({
            "xT": xT,
            "wqkvT": np.ascontiguousarray(wslice.T),
            "bqkv": bslice,
            "wpT": wpT,
            "bp": b_proj,
            "tri": tri,
            "ident": ident,
            "ones": ones,
        })
    return in_maps


def kernel(x, W_qkv, b_qkv, W_proj, b_proj, _trace=False):
    from concourse import bass_utils

    if "nc" not in _CACHE:
        _CACHE["nc"] = _build_nc()
    nc = _CACHE["nc"]
    in_maps = _prep_inputs(x, W_qkv, b_qkv, W_proj, b_proj)
    res = bass_utils.run_bass_kernel_spmd(
        nc, in_maps, core_ids=list(range(NCORES)), trace=_trace,
    )
    _CACHE["last_result"] = res
    # core p rows: [b*256 + i] = batch b, token 256*p + i
    yfull = np.empty((B, T, C), dtype=np.float32)
    for p, rmap in enumerate(res.results):
        yp = rmap["y"]
        for b in range(B):
            yfull[b, SL * p:SL * (p + 1)] = yp[b * SL:(b + 1) * SL]
    return yfull


# revision 24
# speedup vs baseline: 1.4787x; 1.0866x over previous
"""Causal self-attention (B=2, T=2048, C=1024, H=16) on 8 trn2 NeuronCores.

Sharding (Megatron-style, per spec hint):
  - tensor-parallel over heads: core p owns heads {2p, 2p+1}.  Each core
    computes Q^T/K^T/V^T for its 2 heads from the full x, then causal
    attention (streaming softmax without max-subtraction; the denominator
    comes from a ones-column appended to V).
  - per batch: an AllToAll redistributes that batch's attention outputs so
    that core p holds all 1024 channels for the batch's tokens
    [256p, 256p+256); batch 0's AllToAll and projection overlap batch 1's
    qkv/attention work.
  - projection: each core computes the full output projection for its two
    256-token slices and writes a disjoint [512, 1024] output block
    (rows b*256+i = batch b, token 256*p+i).

The emission order pipelines per batch so the (in-order) PE never waits on
the 16.8 MB x^T stream: qkv(b0) -> attention(b0) -> qkv(b1) [x tail
streams during attention(b0)] -> attention(b1) -> projections.

Layouts: x/W pre-transposed on host so contractions land on partitions;
scores computed transposed (S^T = K Q^T) so the softmax sum is a matmul
reduction and exp(S^T) feeds O^T = V^T_aug P directly, accumulated in PSUM
over k-tiles; all matmuls in float32r.
"""

import numpy as np

B, T, C, H, D = 2, 2048, 1024, 16, 64
NCORES = 8
HL = H // NCORES        # heads per core = 2
TOK = B * T             # 4096 global tokens
TSL = TOK // NCORES     # 512 output tokens per core (256 per batch)
SL = 256                # per-batch token slice per core
P = 128
CT = C // P             # 8 contraction tiles
NQC = T // 512          # 4 q-chunks per batch
NKT = T // P            # 16 k-tiles per batch
KG = 2                  # k-tiles per exp group
SCALE = D ** -0.5

_CACHE = {}


def _build_nc():
    import concourse.bass as bass
    import concourse.mybir as mybir
    from concourse import bacc
    from concourse.tile import TileContext

    f32 = mybir.dt.float32
    f32r = mybir.dt.float32r
    bf16 = mybir.dt.bfloat16
    AF = mybir.ActivationFunctionType
    ALU = mybir.AluOpType

    nc = bacc.Bacc(
        "TRN2", target_bir_lowering=False, debug=False, num_devices=NCORES
    )

    xT = nc.dram_tensor("xT", [C, TOK], f32r, kind="ExternalInput")
    wqkvT = nc.dram_tensor("wqkvT", [C, 3 * P], f32r, kind="ExternalInput")
    bqkv = nc.dram_tensor("bqkv", [3 * P], f32, kind="ExternalInput")
    wpT = nc.dram_tensor("wpT", [C, C], f32r, kind="ExternalInput")
    bp = nc.dram_tensor("bp", [C], f32r, kind="ExternalInput")
    tri = nc.dram_tensor("tri", [P, P], bf16, kind="ExternalInput")
    onesd = nc.dram_tensor("ones", [P, P], f32r, kind="ExternalInput")
    ident = nc.dram_tensor("ident", [P, P], f32, kind="ExternalInput")
    y = nc.dram_tensor("y", [TSL, C], f32, kind="ExternalOutput")

    with TileContext(nc, num_cores=NCORES) as tc:
        from contextlib import ExitStack

        with ExitStack() as ctx:
            const = ctx.enter_context(tc.tile_pool(name="const", bufs=1))
            persist = ctx.enter_context(tc.tile_pool(name="persist", bufs=1))
            dram = ctx.enter_context(tc.tile_pool(name="dram", bufs=1, space="DRAM"))

            # ---- constants; small ones first so nothing queues behind bulk
            tri_sb = const.tile([P, P], bf16)
            id_sb = const.tile([P, P], f32)
            bq_sb = const.tile([P, 3], f32)
            bp_sb = const.tile([1, C], f32r)
            ones_sb = const.tile([1, P], f32r)
            ones2_sb = const.tile([P, 2], bf16)
            w_sb = const.tile([P, CT, 3 * P], f32r)     # wqkvT tiles
            wp_sb = const.tile([P, CT, C], f32r)        # W_proj^T (loaded late)
            nc.gpsimd.dma_start(tri_sb[:], tri[:])
            nc.gpsimd.dma_start(id_sb[:], ident[:])
            nc.gpsimd.dma_start(bq_sb[:], bqkv.rearrange("(et p) -> p et", p=P))
            nc.gpsimd.dma_start(bp_sb[:], bp.rearrange("(o c) -> o c", o=1))
            nc.gpsimd.dma_start(ones_sb[:], onesd[0:1, :])
            nc.gpsimd.dma_start(ones2_sb[:], onesd[:, 0:2].bitcast(f32))
            nc.sync.dma_start(w_sb[:], wqkvT.rearrange("(ct p) e -> p ct e", p=P))

            # ---- persistent activations (per batch for fine-grained deps)
            qTb = [persist.tile([P, T], bf16, name=f"qT{b}") for b in range(B)]
            kTb = [persist.tile([P, T], bf16, name=f"kT{b}") for b in range(B)]
            vTb = [persist.tile([P, T], f32, name=f"vT{b}") for b in range(B)]
            # V with ones column, per batch: [128 tok, k-tile, 2*65]
            vaugb = [persist.tile([P, NKT, 2 * 65], bf16, name=f"vaug{b}")
                     for b in range(B)]
            # A^T per local head (each head stays at partitions 0-63)
            anorm = [persist.tile([64, TOK], f32, name=f"anorm{h}")
                     for h in range(HL)]
            ddram = dram.tile([B * HL * NQC, 512], f32)  # raw denominators
            rdram = dram.tile([B * HL, T], f32)          # reciprocals (bounce)

            pools = [
                tc.tile_pool(name="sps", bufs=2, space="PSUM"),
                tc.tile_pool(name="ops", bufs=2, space="PSUM"),
                tc.tile_pool(name="mm", bufs=2, space="PSUM"),
                tc.tile_pool(name="pT", bufs=2),
                tc.tile_pool(name="ds", bufs=2),
                tc.tile_pool(name="rp", bufs=2),
            ]
            sps, ops, mm, ppool, dspool, rppool = (
                ctx.enter_context(p) for p in pools)

            def qkv(b):
                """qkv^T for batch b's 4 token chunks + V transposes."""
                for tc4 in range(4):
                    xsl = xpool.tile([P, CT, 512], f32r, tag="x")
                    t0 = b * T + tc4 * 512
                    nc.sync.dma_start(
                        xsl[:],
                        xT[:, t0:t0 + 512].rearrange("(ct p) t -> p ct t", p=P),
                    )
                    for et, dstl in enumerate((qTb, kTb, vTb)):
                        ps = mm.tile([P, 512], f32, tag="mm")
                        for ct in range(CT):
                            nc.tensor.matmul(
                                ps[:],
                                lhsT=w_sb[:, ct, et * P:(et + 1) * P],
                                rhs=xsl[:, ct, :],
                                start=(ct == 0),
                                stop=(ct == CT - 1),
                            )
                        nc.vector.tensor_scalar_add(
                            dstl[b][:, tc4 * 512:(tc4 + 1) * 512],
                            ps[:],
                            bq_sb[:, et:et + 1],
                        )
                    # V^T -> V for this chunk's 4 k-tiles (PE transpose)
                    for kt in range(tc4 * 4, tc4 * 4 + 4):
                        tp = mm.tile([P, P], f32, tag="mm")
                        nc.tensor.transpose(
                            tp[:],
                            vTb[b][:, kt * P:(kt + 1) * P],
                            id_sb[:],
                        )
                        nc.vector.tensor_copy(
                            vaugb[b][:, kt, 0:2 * 65]
                            .rearrange("p (h e) -> p h e", h=2)[:, :, 0:64],
                            tp.rearrange("p (h e) -> p h e", h=2),
                        )
                        nc.vector.tensor_copy(
                            vaugb[b][:, kt, 64:2 * 65:65], ones2_sb[:]
                        )

            def attention(b):
                for h in range(HL):
                    bh = b * HL + h
                    hp = slice(64 * h, 64 * h + 64)
                    for qc in range(NQC):
                        q0 = qc * 512
                        nk = 4 * qc + 4               # causal k-tiles
                        ops_t = ops.tile([65, 512], f32, tag="o")
                        for g0 in range(0, nk, KG):
                            gn = min(KG, nk - g0)
                            sp = sps.tile([P, KG * 512], f32, tag="s")
                            for j in range(gn):
                                ki = g0 + j
                                nc.tensor.matmul(
                                    sp[:, j * 512:(j + 1) * 512],
                                    lhsT=kTb[b][hp, ki * P:(ki + 1) * P],
                                    rhs=qTb[b][hp, q0:q0 + 512],
                                    start=True,
                                    stop=True,
                                )
                            pt = ppool.tile([P, KG * 512], bf16, tag="p")
                            nc.scalar.activation(
                                pt[:, 0:gn * 512],
                                sp[:, 0:gn * 512],
                                AF.Exp,
                                scale=SCALE,
                            )
                            for j in range(gn):
                                ki = g0 + j
                                off = ki * P - q0
                                if 0 <= off:
                                    nc.vector.tensor_tensor(
                                        pt[:, j * 512 + off:
                                           j * 512 + off + P],
                                        pt[:, j * 512 + off:
                                           j * 512 + off + P],
                                        tri_sb[:],
                                        ALU.mult,
                                    )
                                lo = max(0, off)
                                nc.tensor.matmul(
                                    ops_t[:, lo:512],
                                    lhsT=vaugb[b][:, ki, h * 65:h * 65 + 65],
                                    rhs=pt[:, j * 512 + lo:(j + 1) * 512],
                                    start=(ki == 0),
                                    stop=(ki == nk - 1),
                                )
                        # stash unnormalised O^T rows + denominator row
                        nc.vector.tensor_copy(
                            anorm[h][:, b * T + q0:b * T + q0 + 512],
                            ops_t[0:64, :],
                        )
                        dst = dspool.tile([65, 512], f32, tag="ds")
                        nc.vector.tensor_copy(dst[64:65, :], ops_t[64:65, :])
                        nc.sync.dma_start(
                            ddram[bh * NQC + qc:bh * NQC + qc + 1, :],
                            dst[64:65, :],
                        )

            def normalize(b):
                for h in range(HL):
                    bh = b * HL + h
                    dpk = rppool.tile([32, 64], f32, tag="dpk")
                    rpk = rppool.tile([32, 64], f32, tag="rpk")
                    rsc = rppool.tile([32, 64], f32, tag="rsc")
                    nc.sync.dma_start(
                        dpk[:],
                        ddram[bh * NQC:(bh + 1) * NQC, :]
                        .rearrange("u (rr f) -> (u rr) f", f=64),
                    )
                    nc.vector.reciprocal_approx_accurate(rpk[:], dpk[:], rsc[:])
                    nc.sync.dma_start(
                        rdram[bh:bh + 1, :]
                        .rearrange("o (rr f) -> (o rr) f", f=64),
                        rpk[:],
                    )
                    rb = rbpool.tile([64, T], f32, tag="rb")
                    nc.sync.dma_start(
                        rb[:],
                        rdram[bh:bh + 1, :].to_broadcast((64, T)),
                    )
                    nc.vector.tensor_tensor(
                        anorm[h][:, b * T:(b + 1) * T],
                        anorm[h][:, b * T:(b + 1) * T],
                        rb[:],
                        ALU.mult,
                    )

            def a2a(b):
                a2a_in = dram.tile([NCORES * P, SL], f32r, name=f"a2a_in{b}")
                a2a_out = dram.tile([NCORES * P, SL], f32r, name=f"a2a_out{b}")
                a2a_v = a2a_in.rearrange("(j ee) t -> ee j t", j=NCORES)
                for h in range(HL):
                    nc.sync.dma_start(
                        a2a_v[64 * h:64 * h + 64],
                        anorm[h][:, b * T:(b + 1) * T]
                        .rearrange("e (j t) -> e j t", j=NCORES)
                        .bitcast(f32r),
                    )
                nc.gpsimd.collective_compute(
                    "AllToAll",
                    ALU.bypass,
                    replica_groups=[list(range(NCORES))],
                    ins=[a2a_in.opt()],
                    outs=[a2a_out.opt()],
                )
                return a2a_out

            def proj(b, a2a_out):
                afull = apool.tile([P, NCORES, SL], f32r, tag="af")
                nc.sync.dma_start(
                    afull[:],
                    a2a_out.rearrange("(i e) t -> e i t", i=NCORES),
                )
                for tt in range(SL // P):
                    for fc in range(C // 512):
                        ps = mm.tile([P, 512], f32, tag="mm")
                        nc.tensor.matmul(
                            ps[:],
                            lhsT=ones_sb[:],
                            rhs=bp_sb[:, fc * 512:(fc + 1) * 512],
                            start=True,
                            stop=False,
                        )
                        for i in range(NCORES):
                            nc.tensor.matmul(
                                ps[:],
                                lhsT=afull[:, i, tt * P:(tt + 1) * P],
                                rhs=wp_sb[:, i, fc * 512:(fc + 1) * 512],
                                start=False,
                                stop=(i == NCORES - 1),
                            )
                        ysb = ypool.tile([P, 512], f32, tag="ysb")
                        nc.vector.tensor_copy(ysb[:], ps[:])
                        nc.sync.dma_start(
                            y[b * SL + tt * P:b * SL + (tt + 1) * P,
                              fc * 512:(fc + 1) * 512],
                            ysb[:],
                        )

            with tc.tile_pool(name="xslab", bufs=2) as xpool:
                qkv(0)
                attention(0)
                qkv(1)
            with tc.tile_pool(name="rb", bufs=1) as rbpool, \
                 tc.tile_pool(name="afull", bufs=2) as apool, \
                 tc.tile_pool(name="ysb", bufs=2) as ypool:
                normalize(0)
                out0 = a2a(0)
                nc.sync.dma_start(
                    wp_sb[:], wpT.rearrange("(ct p) f -> p ct f", p=P)
                )
                attention(1)
                proj(0, out0)
                normalize(1)
                out1 = a2a(1)
                proj(1, out1)
    nc.compile()
    return nc


def _prep_inputs(x, W_qkv, b_qkv, W_proj, b_proj):
    x = np.asarray(x, dtype=np.float32)
    W_qkv = np.asarray(W_qkv, dtype=np.float32)
    b_qkv = np.asarray(b_qkv, dtype=np.float32)
    W_proj = np.asarray(W_proj, dtype=np.float32)
    b_proj = np.asarray(b_proj, dtype=np.float32)

    xT = np.ascontiguousarray(x.reshape(TOK, C).T)
    wpT = np.ascontiguousarray(W_proj.T)
    import ml_dtypes
    tri = np.triu(np.ones((P, P), dtype=np.float32)).astype(ml_dtypes.bfloat16)
    ident = np.eye(P, dtype=np.float32)
    ones = np.ones((P, P), dtype=np.float32)

    in_maps = []
    for p in range(NCORES):
        rows = np.r_[128 * p:128 * p + 128,
                     C + 128 * p:C + 128 * p + 128,
                     2 * C + 128 * p:2 * C + 128 * p + 128]
        wslice = W_qkv[rows]                      # [384, 1024]
        bslice = np.ascontiguousarray(b_qkv[rows])
        in_maps.append({
            "xT": xT,
            "wqkvT": np.ascontiguousarray(wslice.T),
            "bqkv": bslice,
            "wpT": wpT,
            "bp": b_proj,
            "tri": tri,
            "ident": ident,
            "ones": ones,
        })
    return in_maps


def kernel(x, W_qkv, b_qkv, W_proj, b_proj, _trace=False):
    from concourse import bass_utils

    if "nc" not in _CACHE:
        _CACHE["nc"] = _build_nc()
    nc = _CACHE["nc"]
    in_maps = _prep_inputs(x, W_qkv, b_qkv, W_proj, b_proj)
    res = bass_utils.run_bass_kernel_spmd(
        nc, in_maps, core_ids=list(range(NCORES)), trace=_trace,
    )
    _CACHE["last_result"] = res
    # core p rows: [b*256 + i] = batch b, token 256*p + i
    yfull = np.empty((B, T, C), dtype=np.float32)
    for p, rmap in enumerate(res.results):
        yp = rmap["y"]
        for b in range(B):
            yfull[b, SL * p:SL * (p + 1)] = yp[b * SL:(b + 1) * SL]
    return yfull


# revision 26
# speedup vs baseline: 1.5500x; 1.0482x over previous
"""Causal self-attention (B=2, T=2048, C=1024, H=16) on 8 trn2 NeuronCores.

Sharding (Megatron-style, per spec hint):
  - tensor-parallel over heads: core p owns heads {2p, 2p+1}.  Each core
    computes Q^T/K^T/V^T for its 2 heads from the full x, then causal
    attention (streaming softmax without max-subtraction; the denominator
    comes from a ones-column appended to V).
  - per batch: an AllToAll redistributes that batch's attention outputs so
    that core p holds all 1024 channels for the batch's tokens
    [256p, 256p+256); batch 0's AllToAll and projection overlap batch 1's
    qkv/attention work.
  - projection: each core computes the full output projection for its two
    256-token slices and writes a disjoint [512, 1024] output block
    (rows b*256+i = batch b, token 256*p+i).

The emission order pipelines per batch so the (in-order) PE never waits on
the 16.8 MB x^T stream: qkv(b0) -> attention(b0) -> qkv(b1) [x tail
streams during attention(b0)] -> attention(b1) -> projections.

Layouts: x/W pre-transposed on host so contractions land on partitions;
scores computed transposed (S^T = K Q^T) so the softmax sum is a matmul
reduction and exp(S^T) feeds O^T = V^T_aug P directly, accumulated in PSUM
over k-tiles; all matmuls in float32r.
"""

import numpy as np

B, T, C, H, D = 2, 2048, 1024, 16, 64
NCORES = 8
HL = H // NCORES        # heads per core = 2
TOK = B * T             # 4096 global tokens
TSL = TOK // NCORES     # 512 output tokens per core (256 per batch)
SL = 256                # per-batch token slice per core
P = 128
CT = C // P             # 8 contraction tiles
NQC = T // 512          # 4 q-chunks per batch
NKT = T // P            # 16 k-tiles per batch
KG = 2                  # k-tiles per exp group
SCALE = D ** -0.5

_CACHE = {}


def _build_nc():
    import concourse.bass as bass
    import concourse.mybir as mybir
    from concourse import bacc
    from concourse.tile import TileContext

    f32 = mybir.dt.float32
    f32r = mybir.dt.float32r
    bf16 = mybir.dt.bfloat16
    AF = mybir.ActivationFunctionType
    ALU = mybir.AluOpType

    nc = bacc.Bacc(
        "TRN2", target_bir_lowering=False, debug=False, num_devices=NCORES
    )

    xT = nc.dram_tensor("xT", [C, TOK], bf16, kind="ExternalInput")
    wqkvT = nc.dram_tensor("wqkvT", [C, 3 * P], bf16, kind="ExternalInput")
    bqkv = nc.dram_tensor("bqkv", [3 * P], f32, kind="ExternalInput")
    wpT = nc.dram_tensor("wpT", [C, C], bf16, kind="ExternalInput")
    bp = nc.dram_tensor("bp", [C], bf16, kind="ExternalInput")
    tri = nc.dram_tensor("tri", [P, P], bf16, kind="ExternalInput")
    onesd = nc.dram_tensor("ones", [P, P], bf16, kind="ExternalInput")
    ident = nc.dram_tensor("ident", [P, P], bf16, kind="ExternalInput")
    y = nc.dram_tensor("y", [TSL, C], f32, kind="ExternalOutput")

    with TileContext(nc, num_cores=NCORES) as tc:
        from contextlib import ExitStack

        with ExitStack() as ctx:
            const = ctx.enter_context(tc.tile_pool(name="const", bufs=1))
            persist = ctx.enter_context(tc.tile_pool(name="persist", bufs=1))
            dram = ctx.enter_context(tc.tile_pool(name="dram", bufs=1, space="DRAM"))

            # ---- constants; small ones first so nothing queues behind bulk
            tri_sb = const.tile([P, P], bf16)
            id_sb = const.tile([P, P], bf16)
            bq_sb = const.tile([P, 3], f32)
            bp_sb = const.tile([1, C], bf16)
            ones_sb = const.tile([1, P], bf16)
            ones2_sb = const.tile([P, 2], bf16)
            w_sb = const.tile([P, CT, 3 * P], bf16)     # wqkvT tiles
            wp_sb = const.tile([P, CT, C], bf16)        # W_proj^T (loaded late)
            nc.gpsimd.dma_start(tri_sb[:], tri[:])
            nc.gpsimd.dma_start(id_sb[:], ident[:])
            nc.gpsimd.dma_start(bq_sb[:], bqkv.rearrange("(et p) -> p et", p=P))
            nc.gpsimd.dma_start(bp_sb[:], bp.rearrange("(o c) -> o c", o=1))
            nc.gpsimd.dma_start(ones_sb[:], onesd[0:1, :])
            nc.gpsimd.dma_start(ones2_sb[:], onesd[:, 0:2])
            nc.sync.dma_start(w_sb[:], wqkvT.rearrange("(ct p) e -> p ct e", p=P))

            # ---- persistent activations (per batch for fine-grained deps)
            qTb = [persist.tile([P, T], bf16, name=f"qT{b}") for b in range(B)]
            kTb = [persist.tile([P, T], bf16, name=f"kT{b}") for b in range(B)]
            vTb = [persist.tile([P, T], bf16, name=f"vT{b}") for b in range(B)]
            # V with ones column, per batch: [128 tok, k-tile, 2*65]
            vaugb = [persist.tile([P, NKT, 2 * 65], bf16, name=f"vaug{b}")
                     for b in range(B)]
            # A^T per local head (each head stays at partitions 0-63)
            anorm = [persist.tile([64, TOK], bf16, name=f"anorm{h}")
                     for h in range(HL)]
            ddram = dram.tile([B * HL * NQC, 512], f32)  # raw denominators
            rdram = dram.tile([B * HL, T], f32)          # reciprocals (bounce)

            pools = [
                tc.tile_pool(name="sps", bufs=2, space="PSUM"),
                tc.tile_pool(name="ops", bufs=2, space="PSUM"),
                tc.tile_pool(name="mm", bufs=2, space="PSUM"),
                tc.tile_pool(name="pT", bufs=2),
                tc.tile_pool(name="ds", bufs=2),
                tc.tile_pool(name="rp", bufs=2),
            ]
            sps, ops, mm, ppool, dspool, rppool = (
                ctx.enter_context(p) for p in pools)

            def qkv(b):
                """qkv^T for batch b's 4 token chunks + V transposes."""
                for tc4 in range(4):
                    xsl = xpool.tile([P, CT, 512], bf16, tag="x")
                    t0 = b * T + tc4 * 512
                    nc.sync.dma_start(
                        xsl[:],
                        xT[:, t0:t0 + 512].rearrange("(ct p) t -> p ct t", p=P),
                    )
                    for et, dstl in enumerate((qTb, kTb, vTb)):
                        ps = mm.tile([P, 512], f32, tag="mm")
                        for ct in range(CT):
                            nc.tensor.matmul(
                                ps[:],
                                lhsT=w_sb[:, ct, et * P:(et + 1) * P],
                                rhs=xsl[:, ct, :],
                                start=(ct == 0),
                                stop=(ct == CT - 1),
                            )
                        nc.vector.tensor_scalar_add(
                            dstl[b][:, tc4 * 512:(tc4 + 1) * 512],
                            ps[:],
                            bq_sb[:, et:et + 1],
                        )
                    # V^T -> V for this chunk's 4 k-tiles (PE transpose)
                    for kt in range(tc4 * 4, tc4 * 4 + 4):
                        tp = mm.tile([P, P], bf16, tag="mm")
                        nc.tensor.transpose(
                            tp[:],
                            vTb[b][:, kt * P:(kt + 1) * P],
                            id_sb[:],
                        )
                        nc.vector.tensor_copy(
                            vaugb[b][:, kt, 0:2 * 65]
                            .rearrange("p (h e) -> p h e", h=2)[:, :, 0:64],
                            tp.rearrange("p (h e) -> p h e", h=2),
                        )
                        nc.vector.tensor_copy(
                            vaugb[b][:, kt, 64:2 * 65:65], ones2_sb[:]
                        )

            def attention(b):
                for h in range(HL):
                    bh = b * HL + h
                    hp = slice(64 * h, 64 * h + 64)
                    for qc in range(NQC):
                        q0 = qc * 512
                        nk = 4 * qc + 4               # causal k-tiles
                        ops_t = ops.tile([65, 512], f32, tag="o")
                        for g0 in range(0, nk, KG):
                            gn = min(KG, nk - g0)
                            sp = sps.tile([P, KG * 512], f32, tag="s")
                            for j in range(gn):
                                ki = g0 + j
                                nc.tensor.matmul(
                                    sp[:, j * 512:(j + 1) * 512],
                                    lhsT=kTb[b][hp, ki * P:(ki + 1) * P],
                                    rhs=qTb[b][hp, q0:q0 + 512],
                                    start=True,
                                    stop=True,
                                )
                            pt = ppool.tile([P, KG * 512], bf16, tag="p")
                            nc.scalar.activation(
                                pt[:, 0:gn * 512],
                                sp[:, 0:gn * 512],
                                AF.Exp,
                                scale=SCALE,
                            )
                            for j in range(gn):
                                ki = g0 + j
                                off = ki * P - q0
                                if 0 <= off:
                                    nc.vector.tensor_tensor(
                                        pt[:, j * 512 + off:
                                           j * 512 + off + P],
                                        pt[:, j * 512 + off:
                                           j * 512 + off + P],
                                        tri_sb[:],
                                        ALU.mult,
                                    )
                                lo = max(0, off)
                                nc.tensor.matmul(
                                    ops_t[:, lo:512],
                                    lhsT=vaugb[b][:, ki, h * 65:h * 65 + 65],
                                    rhs=pt[:, j * 512 + lo:(j + 1) * 512],
                                    start=(ki == 0),
                                    stop=(ki == nk - 1),
                                )
                        # stash unnormalised O^T rows + denominator row
                        nc.vector.tensor_copy(
                            anorm[h][:, b * T + q0:b * T + q0 + 512],
                            ops_t[0:64, :],
                        )
                        dst = dspool.tile([65, 512], f32, tag="ds")
                        nc.vector.tensor_copy(dst[64:65, :], ops_t[64:65, :])
                        nc.sync.dma_start(
                            ddram[bh * NQC + qc:bh * NQC + qc + 1, :],
                            dst[64:65, :],
                        )

            def normalize(b):
                for h in range(HL):
                    bh = b * HL + h
                    dpk = rppool.tile([32, 64], f32, tag="dpk")
                    rpk = rppool.tile([32, 64], f32, tag="rpk")
                    rsc = rppool.tile([32, 64], f32, tag="rsc")
                    nc.sync.dma_start(
                        dpk[:],
                        ddram[bh * NQC:(bh + 1) * NQC, :]
                        .rearrange("u (rr f) -> (u rr) f", f=64),
                    )
                    nc.vector.reciprocal_approx_accurate(rpk[:], dpk[:], rsc[:])
                    nc.sync.dma_start(
                        rdram[bh:bh + 1, :]
                        .rearrange("o (rr f) -> (o rr) f", f=64),
                        rpk[:],
                    )
                    rb = rbpool.tile([64, T], f32, tag="rb")
                    nc.sync.dma_start(
                        rb[:],
                        rdram[bh:bh + 1, :].to_broadcast((64, T)),
                    )
                    nc.vector.tensor_tensor(
                        anorm[h][:, b * T:(b + 1) * T],
                        anorm[h][:, b * T:(b + 1) * T],
                        rb[:],
                        ALU.mult,
                    )

            def a2a(b):
                a2a_in = dram.tile([NCORES * P, SL], bf16, name=f"a2a_in{b}")
                a2a_out = dram.tile([NCORES * P, SL], bf16, name=f"a2a_out{b}")
                a2a_v = a2a_in.rearrange("(j ee) t -> ee j t", j=NCORES)
                for h in range(HL):
                    nc.sync.dma_start(
                        a2a_v[64 * h:64 * h + 64],
                        anorm[h][:, b * T:(b + 1) * T]
                        .rearrange("e (j t) -> e j t", j=NCORES),
                    )
                nc.gpsimd.collective_compute(
                    "AllToAll",
                    ALU.bypass,
                    replica_groups=[list(range(NCORES))],
                    ins=[a2a_in.opt()],
                    outs=[a2a_out.opt()],
                )
                return a2a_out

            def proj(b, a2a_out):
                afull = apool.tile([P, NCORES, SL], bf16, tag="af")
                nc.sync.dma_start(
                    afull[:],
                    a2a_out.rearrange("(i e) t -> e i t", i=NCORES),
                )
                for tt in range(SL // P):
                    for fc in range(C // 512):
                        ps = mm.tile([P, 512], f32, tag="mm")
                        nc.tensor.matmul(
                            ps[:],
                            lhsT=ones_sb[:],
                            rhs=bp_sb[:, fc * 512:(fc + 1) * 512],
                            start=True,
                            stop=False,
                        )
                        for i in range(NCORES):
                            nc.tensor.matmul(
                                ps[:],
                                lhsT=afull[:, i, tt * P:(tt + 1) * P],
                                rhs=wp_sb[:, i, fc * 512:(fc + 1) * 512],
                                start=False,
                                stop=(i == NCORES - 1),
                            )
                        ysb = ypool.tile([P, 512], f32, tag="ysb")
                        nc.vector.tensor_copy(ysb[:], ps[:])
                        nc.sync.dma_start(
                            y[b * SL + tt * P:b * SL + (tt + 1) * P,
                              fc * 512:(fc + 1) * 512],
                            ysb[:],
                        )

            with tc.tile_pool(name="xslab", bufs=2) as xpool:
                qkv(0)
                attention(0)
                qkv(1)
            with tc.tile_pool(name="rb", bufs=1) as rbpool, \
                 tc.tile_pool(name="afull", bufs=2) as apool, \
                 tc.tile_pool(name="ysb", bufs=2) as ypool:
                normalize(0)
                out0 = a2a(0)
                nc.sync.dma_start(
                    wp_sb[:], wpT.rearrange("(ct p) f -> p ct f", p=P)
                )
                attention(1)
                proj(0, out0)
                normalize(1)
                out1 = a2a(1)
                proj(1, out1)
    nc.compile()
    return nc


def _prep_inputs(x, W_qkv, b_qkv, W_proj, b_proj):
    x = np.asarray(x, dtype=np.float32)
    W_qkv = np.asarray(W_qkv, dtype=np.float32)
    b_qkv = np.asarray(b_qkv, dtype=np.float32)
    W_proj = np.asarray(W_proj, dtype=np.float32)
    b_proj = np.asarray(b_proj, dtype=np.float32)

    import ml_dtypes
    bf = ml_dtypes.bfloat16
    xT = np.ascontiguousarray(x.reshape(TOK, C).T).astype(bf)
    wpT = np.ascontiguousarray(W_proj.T).astype(bf)
    tri = np.triu(np.ones((P, P), dtype=np.float32)).astype(bf)
    ident = np.eye(P, dtype=np.float32).astype(bf)
    ones = np.ones((P, P), dtype=np.float32).astype(bf)

    in_maps = []
    for p in range(NCORES):
        rows = np.r_[128 * p:128 * p + 128,
                     C + 128 * p:C + 128 * p + 128,
                     2 * C + 128 * p:2 * C + 128 * p + 128]
        wslice = W_qkv[rows]                      # [384, 1024]
        bslice = np.ascontiguousarray(b_qkv[rows])
        in_maps.append({
            "xT": xT,
            "wqkvT": np.ascontiguousarray(wslice.T).astype(bf),
            "bqkv": bslice,
            "wpT": wpT,
            "bp": b_proj.astype(bf),
            "tri": tri,
            "ident": ident,
            "ones": ones,
        })
    return in_maps


def kernel(x, W_qkv, b_qkv, W_proj, b_proj, _trace=False):
    from concourse import bass_utils

    if "nc" not in _CACHE:
        _CACHE["nc"] = _build_nc()
    nc = _CACHE["nc"]
    in_maps = _prep_inputs(x, W_qkv, b_qkv, W_proj, b_proj)
    res = bass_utils.run_bass_kernel_spmd(
        nc, in_maps, core_ids=list(range(NCORES)), trace=_trace,
    )
    _CACHE["last_result"] = res
    # core p rows: [b*256 + i] = batch b, token 256*p + i
    yfull = np.empty((B, T, C), dtype=np.float32)
    for p, rmap in enumerate(res.results):
        yp = rmap["y"]
        for b in range(B):
            yfull[b, SL * p:SL * (p + 1)] = yp[b * SL:(b + 1) * SL]
    return yfull
